# revision 12
# baseline (speedup 1.0000x reference)
"""PhaseEncoding kernel for Trainium2 (8-core SPMD).

Math: out[b,d,s] = x[b,d,s] + sum_f phase_one_hot[b,f,s] * emb_weight[f,d]
Shapes: x (16,512,4096) f32, phase_one_hot (16,9,4096) f32, emb_weight (9,512) f32.
Sharding: batch data-parallel, 2 batches per core; emb_weight replicated.
"""

import numpy as np

B, F, S, D = 16, 9, 4096, 512
NCORES = 8
BPC = B // NCORES  # batches per core

_NC = None


def _build_nc():
    from contextlib import ExitStack

    import concourse.bass as bass
    import concourse.tile as tile
    from concourse import bacc, mybir

    f32 = mybir.dt.float32
    f32r = mybir.dt.float32r
    nc = bacc.Bacc(
        "TRN2", target_bir_lowering=False, debug=False, num_devices=NCORES
    )

    x_d = nc.declare_dram_parameter("x", [BPC, D, S], f32, isOutput=False)
    poh_d = nc.declare_dram_parameter("phase_one_hot", [BPC, F, S], f32, isOutput=False)
    whi_d = nc.declare_dram_parameter("emb_hi", [F, D], f32, isOutput=False)
    wlo_d = nc.declare_dram_parameter("emb_lo", [F, D], f32, isOutput=False)
    out_d = nc.declare_dram_parameter("out", [BPC, D, S], f32, isOutput=True)

    DC = D // 128  # 4 d-chunks of 128 partitions
    ST = S // 512  # 8 s-tiles of 512 columns

    with tile.TileContext(nc) as tc, ExitStack() as ctx:
        const_pool = ctx.enter_context(tc.tile_pool(name="const", bufs=1))
        poh_pool = ctx.enter_context(tc.tile_pool(name="poh", bufs=2))
        x_pool = ctx.enter_context(tc.tile_pool(name="x", bufs=4))
        o_pool = ctx.enter_context(tc.tile_pool(name="o", bufs=3))
        psum_pool = ctx.enter_context(
            tc.tile_pool(name="psum", bufs=8, space=bass.MemorySpace.PSUM)
        )

        whi_t = const_pool.tile([F, D], f32r)
        nc.sync.dma_start(whi_t[:], whi_d[:].bitcast(f32r))
        wlo_t = const_pool.tile([F, D], f32r)
        nc.sync.dma_start(wlo_t[:], wlo_d[:].bitcast(f32r))

        for b in range(BPC):
            poh_t = poh_pool.tile([F, S], f32r)
            nc.sync.dma_start(poh_t[:], poh_d[b].bitcast(f32r))
            for dc in range(DC):
                x_t = x_pool.tile([128, S], f32)
                nc.sync.dma_start(x_t[:], x_d[b, bass.ts(dc, 128), :])
                o_t = o_pool.tile([128, S], f32)
                for st in range(ST):
                    ps = psum_pool.tile([128, 512], f32)
                    nc.tensor.matmul(
                        ps[:],
                        whi_t[:, bass.ts(dc, 128)],
                        poh_t[:, bass.ts(st, 512)],
                        start=True,
                        stop=False,
                    )
                    nc.tensor.matmul(
                        ps[:],
                        wlo_t[:, bass.ts(dc, 128)],
                        poh_t[:, bass.ts(st, 512)],
                        start=False,
                        stop=True,
                    )
                    nc.vector.tensor_add(
                        o_t[:, bass.ts(st, 512)],
                        x_t[:, bass.ts(st, 512)],
                        ps[:],
                    )
                nc.gpsimd.dma_start(out_d[b, bass.ts(dc, 128), :], o_t[:])

    nc.compile()
    return nc


def _get_nc():
    global _NC
    if _NC is None:
        _NC = _build_nc()
    return _NC


def kernel(**inputs):
    from concourse.bass_utils import run_bass_kernel_spmd

    x = np.ascontiguousarray(inputs["x"], dtype=np.float32)
    poh = np.ascontiguousarray(inputs["phase_one_hot"], dtype=np.float32)
    w = np.ascontiguousarray(inputs["emb_weight"], dtype=np.float32)

    # Split w so both halves are exact in fp32r (TF32-like 10-bit mantissa):
    # w_hi keeps the top mantissa bits, w_lo the remainder. The two
    # accumulating matmuls then reconstruct w to ~2^-21 relative error.
    w_hi = (w.view(np.uint32) & np.uint32(0xFFFFE000)).view(np.float32)
    w_lo = np.ascontiguousarray(w - w_hi)
    w_hi = np.ascontiguousarray(w_hi)

    nc = _get_nc()
    in_maps = [
        {
            "x": x[i * BPC : (i + 1) * BPC],
            "phase_one_hot": poh[i * BPC : (i + 1) * BPC],
            "emb_hi": w_hi,
            "emb_lo": w_lo,
        }
        for i in range(NCORES)
    ]
    res = run_bass_kernel_spmd(nc, in_maps, core_ids=list(range(NCORES)))
    out = np.concatenate(
        [np.asarray(res.results[i]["out"]) for i in range(NCORES)], axis=0
    )
    return out.astype(np.float32, copy=False)
